# revision 6
# baseline (speedup 1.0000x reference)
"""Trainium2 Bass kernel for batched per-item GRU cell.

Problem: nn_GRU_Cell — B=16, N=207 independent items, each with its own
C=64 -> 3H=192 weight matrices (Wx, Wh).  All ops are per-(b,n):

    xW          = x @ Wx                      [1, 192]
    r           = sigmoid(xW_r + h @ Wh_r + b_r)
    z           = sigmoid(xW_z + h @ Wh_z + b_z)
    hc          = tanh  (xW_c + (r*h) @ Wh_c + b_c)
    h_new       = (1 - z) * h + z * hc

Strategy (per core, items sharded 3312 -> 8 x 414):
  * Weights stream once from HBM in bf16 (20.3MB/core ~ 57us at the
    358GB/s per-NC HBM limit — the roofline).  Per item they are the PE
    *stationary* operand, K-stacked:
      S_rz = [Wx[:, 0:128] ; Wh[:, 0:128]]  (K=128, M=128)
      S_c  = [Wx[:,128:192]; Wh[:,128:192]] (K=128, M=64)
    moving operand is a single bf16 column ([x;h], then [x;r*h]).
    Outputs land as dense PSUM columns [j, item] -> cheap eviction.
  * PE cost is LDWEIGHTS-column-bound (192 cols/item ~ 46us/core) —
    just under the DMA roofline, so the kernel must keep DMA streaming
    continuously and let the PE trail it.
  * DMA orchestration (the critical part):
      - ALL weight DMAs issue from the SP/sync HWDGE queue, which
        carries nothing else, so a dma_start is never stuck behind a
        semaphore-gated compute instruction.
      - Each chunk's weights arrive as sub-DMAs of <=48 items so the
        PE starts on a chunk when its first half lands (Tile tracks
        sub-tile ranges), keeping the PE ~2.7us behind DMA instead of
        a full chunk.
      - aux (x|h|b, packed [128, nchunk*320]) goes as ONE HWDGE DMA on
        the scalar queue up front (SWDGE was measured 3x slower here
        and sat on the critical path).
      - output stores go on the SWDGE/gpsimd path, off both HWDGE
        queues.
  * Chunk schedule [96,96,96,96,30]: big chunks amortize, the small
    tail chunk shortens the final PE+epilogue drain after the last
    DMA byte.
"""

import numpy as np

import concourse.bass as bass
import concourse.mybir as mybir
import concourse.tile as tile
from concourse import bacc
from concourse.bass_utils import run_bass_kernel_spmd
from concourse.masks import make_identity

F32 = mybir.dt.float32
BF16 = mybir.dt.bfloat16

B, N, C, H = 16, 207, 64, 64
J = 3 * H                  # 192
ITEMS = B * N              # 3312
NCORES = 8
PER = ITEMS // NCORES      # 414
CHUNKS = [96, 96, 96, 96, 14, 16]   # sum = 414; tiny tail chunks shorten
                                    # the post-DMA drain
NCHUNK = len(CHUNKS)
GMAX = max(CHUNKS)
SUB = 48                   # weight sub-DMA granularity (items)
AUXW = 2 * C + J           # 320 f32 per item (x | h | b)

AF = mybir.ActivationFunctionType


def build_nc(wdt=BF16, mdt=BF16):
    """Build the per-core Bass program.

    wdt: dtype of the streamed weights (DMA volume / LDW speed).
    mdt: dtype of the moving operand columns (must pair with wdt for PE).
    """
    # Bacc (not raw Bass): its compile() runs move_matmul_waits_to_ldweights
    # + generate_event_semaphores, which split multi-waits down to the 1-wait
    # ISA limit of PE instructions.
    nc = bacc.Bacc(None)
    # aux is host-packed [128, nchunk*320]: chunk k's item p lives at
    # partition p, free range [k*320, (k+1)*320) = x(64) | h(64) | b(192).
    aux_d = nc.declare_dram_parameter("aux", [128, NCHUNK * AUXW], F32,
                                      isOutput=False)
    # weights arrive host-pre-transposed to per-chunk [c, item, j] blocks
    # (flattened): each sub-DMA reads one contiguous run per partition
    w_d = nc.declare_dram_parameter("wxh", [PER * 2 * C * J], wdt,
                                    isOutput=False)
    out_d = nc.declare_dram_parameter("out", [PER, H], F32, isOutput=True)

    cast_rhs = mdt != F32

    with tile.TileContext(nc) as tc:
        with (
            tc.tile_pool(name="const", bufs=1) as cpool,
            tc.tile_pool(name="w", bufs=3) as wpool,
            tc.tile_pool(name="stage", bufs=2) as spool,
            tc.tile_pool(name="act", bufs=2) as apool,
            tc.tile_pool(name="prep", bufs=2, space="PSUM") as prep_pool,
            tc.tile_pool(name="prz", bufs=2, space="PSUM") as prz_pool,
            tc.tile_pool(name="pc", bufs=2, space="PSUM") as pc_pool,
            tc.tile_pool(name="pt", bufs=2, space="PSUM") as pt_pool,
        ):
            ident = cpool.tile([128, 128], F32)
            make_identity(nc, ident[:])

            # aux slices stream on the same sync queue, each slice issued
            # just ahead of its chunk's weights: ~0.4us each, never fights
            # the weight stream for SDMA bandwidth from another queue.
            aux_all = cpool.tile([128, NCHUNK * AUXW], F32)

            s = 0
            woff = 0
            for k in range(NCHUNK):
                G = CHUNKS[k]
                blk = k * AUXW

                # ---- this chunk's x/h/b, then its weights (sync queue) ---
                nc.sync.dma_start(
                    out=aux_all[0:G, blk:blk + AUXW],
                    in_=aux_d[0:G, blk:blk + AUXW],
                )
                # w[c(0:64) | c(64:128), item, j] = [Wx ; Wh]
                w = wpool.tile([128, GMAX, J], wdt, tag="w")
                wsrc = w_d[woff:woff + 128 * G * J].rearrange(
                    "(c g j) -> c g j", c=128, g=G)
                for a in range(0, G, SUB):
                    bnd = min(a + SUB, G)
                    nc.sync.dma_start(
                        out=w[:, a:bnd, :], in_=wsrc[:, a:bnd, :],
                    )

                # ---- transpose x/h and bias to [j, items] ----------------
                txh = aux_all[0:G, blk:blk + 128]
                tb = aux_all[0:G, blk + 128:blk + 128 + J]
                p_xh = prep_pool.tile([128, G], F32, tag="prep")
                nc.tensor.transpose(p_xh[:], txh[:], ident[0:G, 0:G])
                # xh: rows 0:64 = x.T, rows 64:128 = h.T   (f32 master copy)
                xh = apool.tile([128, G], F32, tag="xh")
                nc.scalar.activation(xh[:], p_xh[:], AF.Copy)
                if cast_rhs:
                    xh_m = apool.tile([128, G], mdt, tag="xh_m")
                    nc.vector.tensor_copy(xh_m[:], xh[:])
                else:
                    xh_m = xh
                # c-pass moving columns: x half never changes, fill it now
                # (off the rz->sigmoid->r*h critical chain)
                rhs2 = apool.tile([128, G], mdt, tag="rhs2")
                nc.vector.tensor_copy(rhs2[0:64, :], xh_m[0:64, :])

                p_b = prep_pool.tile([128, G], F32, tag="prep")
                nc.tensor.transpose(p_b[:], tb[:, 0:128], ident[0:G, 0:G])
                b_rz = apool.tile([128, G], F32, tag="b_rz")
                nc.scalar.activation(b_rz[:], p_b[:], AF.Copy)
                p_bc = prep_pool.tile([128, G], F32, tag="prep")
                nc.tensor.transpose(p_bc[0:64, :], tb[:, 128:192], ident[0:G, 0:G])
                b_c = apool.tile([128, G], F32, tag="b_c")
                nc.scalar.activation(b_c[0:64, :], p_bc[0:64, :], AF.Copy)

                # ---- pass 1: per-item rz matmul --------------------------
                psum_rz = prz_pool.tile([128, G], F32, tag="rz")
                for g in range(G):
                    nc.tensor.matmul(
                        psum_rz[:, g:g + 1],
                        w[:, g, 0:128],
                        xh_m[:, g:g + 1],
                        start=True, stop=True,
                    )

                # ---- epilogue 1: r, z, and the c-pass moving columns -----
                t_rz = apool.tile([128, G], F32, tag="t_rz")
                nc.vector.tensor_add(t_rz[:], psum_rz[:], b_rz[:])
                # r evicted to rows 64:128 so r*h aligns with h there
                rs = apool.tile([128, G], F32, tag="rs")
                nc.scalar.activation(rs[64:128, :], t_rz[0:64, :], AF.Sigmoid)
                zs = apool.tile([128, G], F32, tag="zs")
                nc.scalar.activation(zs[64:128, :], t_rz[64:128, :], AF.Sigmoid)
                nc.vector.tensor_mul(rhs2[64:128, :], rs[64:128, :], xh[64:128, :])

                # ---- pass 2: per-item c matmul (xW_c + (r*h) @ Wh_c) -----
                psum_c = pc_pool.tile([128, G], F32, tag="c")
                for g in range(G):
                    nc.tensor.matmul(
                        psum_c[0:64, g:g + 1],
                        w[:, g, 128:192],
                        rhs2[:, g:g + 1],
                        start=True, stop=True,
                    )

                # ---- epilogue 2: hc, h_new = h + z*(hc - h) --------------
                t_c = apool.tile([128, G], F32, tag="t_c")
                nc.vector.tensor_add(t_c[0:64, :], psum_c[0:64, :], b_c[0:64, :])
                # cross-offset ACT move puts hc on 64:128 where z and h live
                hc = apool.tile([128, G], F32, tag="hc")
                nc.scalar.activation(hc[64:128, :], t_c[0:64, :], AF.Tanh)
                e = apool.tile([128, G], F32, tag="e")
                nc.vector.tensor_sub(e[64:128, :], hc[64:128, :], xh[64:128, :])
                f = apool.tile([128, G], F32, tag="f")
                nc.vector.tensor_mul(f[64:128, :], zs[64:128, :], e[64:128, :])
                hn = apool.tile([128, G], F32, tag="hn")
                nc.vector.tensor_add(hn[64:128, :], xh[64:128, :], f[64:128, :])

                # ---- transpose back to [items, H] and store --------------
                p_t = pt_pool.tile([128, 64], F32, tag="t")
                nc.tensor.transpose(
                    p_t[0:G, :], hn[64:128, 0:G], ident[64:128, 64:128]
                )
                ot = spool.tile([GMAX, 64], F32, tag="ot")
                nc.scalar.activation(ot[0:G, :], p_t[0:G, :], AF.Copy)
                if k == NCHUNK - 1:
                    # final store: HWDGE (scalar) — lower fixed cost on the
                    # critical tail, and nothing follows it on that queue
                    nc.scalar.dma_start(out=out_d[s:s + G], in_=ot[0:G, :])
                else:
                    # SWDGE store: keeps both HWDGE queues free for streaming
                    nc.gpsimd.dma_start(out=out_d[s:s + G], in_=ot[0:G, :])

                s += G
                woff += 128 * G * J

    nc.compile()
    return nc


_CACHE = {}


def _get_nc(wdt, mdt):
    key = (wdt, mdt)
    if key not in _CACHE:
        _CACHE[key] = build_nc(wdt, mdt)
    return _CACHE[key]


def _shards(x, state, Wx, Wh, b, wdt_np):
    x2 = np.asarray(x, np.float32).reshape(ITEMS, C)
    h2 = np.asarray(state, np.float32).reshape(ITEMS, H)
    b2 = np.asarray(b, np.float32).reshape(ITEMS, J)
    aux2 = np.ascontiguousarray(np.concatenate([x2, h2, b2], axis=1))
    wx2 = np.asarray(Wx).reshape(ITEMS, C, J)
    wh2 = np.asarray(Wh).reshape(ITEMS, H, J)
    w2 = np.concatenate([wx2, wh2], axis=1).astype(wdt_np)
    w2 = w2.reshape(NCORES, PER, 2 * C, J)
    aux3 = aux2.reshape(NCORES, PER, AUXW)
    maps = []
    for i in range(NCORES):
        # aux packed per chunk: [128 partitions, nchunk * 320]
        auxp = np.zeros((128, NCHUNK * AUXW), np.float32)
        s = 0
        for k, G in enumerate(CHUNKS):
            auxp[0:G, k * AUXW:(k + 1) * AUXW] = aux3[i, s:s + G]
            s += G
        # per chunk: [items, c, j] -> [c, item-in-chunk, j], flattened
        blocks = []
        s = 0
        for G in CHUNKS:
            blocks.append(w2[i, s:s + G].transpose(1, 0, 2).ravel())
            s += G
        maps.append({"aux": auxp, "wxh": np.concatenate(blocks)})
    return maps


def kernel(x, state, Wx, Wh, b, _trace=False, _wdt=BF16, _mdt=BF16):
    import ml_dtypes
    wdt_np = np.float32 if _wdt == F32 else ml_dtypes.bfloat16
    nc = _get_nc(_wdt, _mdt)
    in_maps = _shards(x, state, Wx, Wh, b, wdt_np)
    res = run_bass_kernel_spmd(nc, in_maps, list(range(NCORES)), trace=_trace)
    out = np.concatenate([res.results[i]["out"] for i in range(NCORES)], axis=0)
    ret = out.reshape(B, N, 1, H).astype(np.float32)
    if _trace:
        return ret, res
    return ret


# revision 7
# speedup vs baseline: 1.0565x; 1.0565x over previous
"""Trainium2 Bass kernel for batched per-item GRU cell.

Problem: nn_GRU_Cell — B=16, N=207 independent items, each with its own
C=64 -> 3H=192 weight matrices (Wx, Wh).  All ops are per-(b,n):

    xW          = x @ Wx                      [1, 192]
    r           = sigmoid(xW_r + h @ Wh_r + b_r)
    z           = sigmoid(xW_z + h @ Wh_z + b_z)
    hc          = tanh  (xW_c + (r*h) @ Wh_c + b_c)
    h_new       = (1 - z) * h + z * hc

Strategy (per core, items sharded 3312 -> 8 x 414):
  * Weights stream once from HBM in bf16 (20.3MB/core ~ 59us at the
    ~345GB/s/NC HBM practical limit — the roofline; measured 343).
    Per item they are the PE *stationary* operand, K-stacked:
      S_rz = [Wx[:, 0:128] ; Wh[:, 0:128]]  (K=128, M=128)
      S_c  = [Wx[:,128:192]; Wh[:,128:192]] (K=128, M=64)
    moving operand is a single bf16 column ([x;h], then [x;r*h]).
    Outputs land as dense PSUM columns [j, item] -> cheap eviction.
  * DMA orchestration:
      - ALL input streaming on the SP/sync HWDGE queue, which carries
        nothing else: aux slice (x|h|b) for chunk k, then chunk k's
        weights as sub-DMAs of <=48 items.  One continuous burst.
      - output stores on SWDGE/gpsimd (last one on scalar HWDGE).
  * The compute pipeline is sub-granular (48 items): rz matmuls for a
    sub, its +bias/sigmoid/r*h epilogue, then its c matmuls — so the
    PE trails the DMA stream by only ~one sub, not a whole chunk, and
    almost nothing dangles after the last DMA byte.
  * h_new is produced as [h, items] on partitions 64:128 and stored
    as-is; the host does the final [64, G] -> [G, 64] transpose during
    unsharding (layout-only).
"""

import numpy as np

import concourse.bass as bass
import concourse.mybir as mybir
import concourse.tile as tile
from concourse import bacc
from concourse.bass_utils import run_bass_kernel_spmd
from concourse.masks import make_identity

F32 = mybir.dt.float32
BF16 = mybir.dt.bfloat16

B, N, C, H = 16, 207, 64, 64
J = 3 * H                  # 192
ITEMS = B * N              # 3312
NCORES = 8
PER = ITEMS // NCORES      # 414
CHUNKS = [96, 96, 96, 96, 14, 16]   # sum = 414; tiny tail chunks shorten
                                    # the post-DMA drain
NCHUNK = len(CHUNKS)
GMAX = max(CHUNKS)
SUB = 48                   # sub-granule (weight DMA + compute pipeline)
AUXW = 2 * C + J           # 320 f32 per item (x | h | b)

AF = mybir.ActivationFunctionType


def build_nc(wdt=BF16, mdt=BF16):
    """Build the per-core Bass program.

    wdt: dtype of the streamed weights (DMA volume / LDW speed).
    mdt: dtype of the moving operand columns (must pair with wdt for PE).
    """
    # Bacc (not raw Bass): its compile() runs move_matmul_waits_to_ldweights
    # + generate_event_semaphores, which split multi-waits down to the 1-wait
    # ISA limit of PE instructions.
    nc = bacc.Bacc(None)
    # aux is host-packed [128, nchunk*320]: chunk k's item p lives at
    # partition p, free range [k*320, (k+1)*320) = x(64) | h(64) | b(192).
    aux_d = nc.declare_dram_parameter("aux", [128, NCHUNK * AUXW], F32,
                                      isOutput=False)
    # weights arrive host-pre-transposed to per-chunk [c, item, j] blocks
    # (flattened): each sub-DMA reads one contiguous run per partition
    w_d = nc.declare_dram_parameter("wxh", [PER * 2 * C * J], wdt,
                                    isOutput=False)
    # h_new, chunk-packed as [chunk, h, item-in-chunk] (host transposes)
    out_d = nc.declare_dram_parameter("out", [NCHUNK, H, GMAX], F32,
                                      isOutput=True)

    cast_rhs = mdt != F32

    with tile.TileContext(nc) as tc:
        with (
            tc.tile_pool(name="const", bufs=1) as cpool,
            tc.tile_pool(name="w", bufs=3) as wpool,
            tc.tile_pool(name="act", bufs=2) as apool,
            tc.tile_pool(name="prep", bufs=2, space="PSUM") as prep_pool,
            tc.tile_pool(name="prz", bufs=2, space="PSUM") as prz_pool,
            tc.tile_pool(name="pc", bufs=2, space="PSUM") as pc_pool,
        ):
            ident = cpool.tile([128, 128], F32)
            make_identity(nc, ident[:])

            aux_all = cpool.tile([128, NCHUNK * AUXW], F32)

            woff = 0
            for k in range(NCHUNK):
                G = CHUNKS[k]
                blk = k * AUXW

                # ---- this chunk's x/h/b, then its weights (sync queue) ---
                nc.sync.dma_start(
                    out=aux_all[0:G, blk:blk + AUXW],
                    in_=aux_d[0:G, blk:blk + AUXW],
                )
                # w[c(0:64) | c(64:128), item, j] = [Wx ; Wh]
                w = wpool.tile([128, GMAX, J], wdt, tag="w")
                wsrc = w_d[woff:woff + 128 * G * J].rearrange(
                    "(c g j) -> c g j", c=128, g=G)
                for a in range(0, G, SUB):
                    bnd = min(a + SUB, G)
                    nc.sync.dma_start(
                        out=w[:, a:bnd, :], in_=wsrc[:, a:bnd, :],
                    )

                # ---- transpose x/h and bias to [j, items] ----------------
                txh = aux_all[0:G, blk:blk + 128]
                tb = aux_all[0:G, blk + 128:blk + 128 + J]
                p_xh = prep_pool.tile([128, G], F32, tag="prep")
                nc.tensor.transpose(p_xh[:], txh[:], ident[0:G, 0:G])
                # xh: rows 0:64 = x.T, rows 64:128 = h.T   (f32 master copy)
                xh = apool.tile([128, G], F32, tag="xh")
                nc.scalar.activation(xh[:], p_xh[:], AF.Copy)
                if cast_rhs:
                    xh_m = apool.tile([128, G], mdt, tag="xh_m")
                    nc.vector.tensor_copy(xh_m[:], xh[:])
                else:
                    xh_m = xh
                # c-pass moving columns: x half never changes, fill it now
                # (off the rz->sigmoid->r*h critical chain)
                rhs2 = apool.tile([128, G], mdt, tag="rhs2")
                nc.vector.tensor_copy(rhs2[0:64, :], xh_m[0:64, :])

                p_b = prep_pool.tile([128, G], F32, tag="prep")
                nc.tensor.transpose(p_b[:], tb[:, 0:128], ident[0:G, 0:G])
                b_rz = apool.tile([128, G], F32, tag="b_rz")
                nc.scalar.activation(b_rz[:], p_b[:], AF.Copy)
                p_bc = prep_pool.tile([128, G], F32, tag="prep")
                nc.tensor.transpose(p_bc[0:64, :], tb[:, 128:192], ident[0:G, 0:G])
                b_c = apool.tile([128, G], F32, tag="b_c")
                nc.scalar.activation(b_c[0:64, :], p_bc[0:64, :], AF.Copy)

                psum_rz = prz_pool.tile([128, G], F32, tag="rz")
                psum_c = pc_pool.tile([128, G], F32, tag="c")
                t_rz = apool.tile([128, G], F32, tag="t_rz")
                rs = apool.tile([128, G], F32, tag="rs")
                zs = apool.tile([128, G], F32, tag="zs")
                t_c = apool.tile([128, G], F32, tag="t_c")
                hc = apool.tile([128, G], F32, tag="hc")
                e = apool.tile([128, G], F32, tag="e")
                f = apool.tile([128, G], F32, tag="f")
                hn = apool.tile([128, G], F32, tag="hn")

                # ---- sub-granular pipeline: rz -> gates -> c -------------
                for a in range(0, G, SUB):
                    bb = min(a + SUB, G)
                    for g in range(a, bb):
                        nc.tensor.matmul(
                            psum_rz[:, g:g + 1],
                            w[:, g, 0:128],
                            xh_m[:, g:g + 1],
                            start=True, stop=True,
                        )
                    nc.vector.tensor_add(
                        t_rz[:, a:bb], psum_rz[:, a:bb], b_rz[:, a:bb])
                    # r evicted to rows 64:128 so r*h aligns with h there
                    nc.scalar.activation(
                        rs[64:128, a:bb], t_rz[0:64, a:bb], AF.Sigmoid)
                    nc.scalar.activation(
                        zs[64:128, a:bb], t_rz[64:128, a:bb], AF.Sigmoid)
                    nc.vector.tensor_mul(
                        rhs2[64:128, a:bb], rs[64:128, a:bb], xh[64:128, a:bb])
                    for g in range(a, bb):
                        nc.tensor.matmul(
                            psum_c[0:64, g:g + 1],
                            w[:, g, 128:192],
                            rhs2[:, g:g + 1],
                            start=True, stop=True,
                        )
                    # ---- epilogue: hc, h_new = h + z*(hc - h) ------------
                    nc.vector.tensor_add(
                        t_c[0:64, a:bb], psum_c[0:64, a:bb], b_c[0:64, a:bb])
                    nc.scalar.activation(
                        hc[64:128, a:bb], t_c[0:64, a:bb], AF.Tanh)
                    nc.vector.tensor_sub(
                        e[64:128, a:bb], hc[64:128, a:bb], xh[64:128, a:bb])
                    nc.vector.tensor_mul(
                        f[64:128, a:bb], zs[64:128, a:bb], e[64:128, a:bb])
                    nc.vector.tensor_add(
                        hn[64:128, a:bb], xh[64:128, a:bb], f[64:128, a:bb])

                # ---- store h_new as [h, items]; host transposes ----------
                if k == NCHUNK - 1:
                    # final store: HWDGE (scalar) — lower fixed cost on the
                    # critical tail, and nothing follows it on that queue
                    nc.scalar.dma_start(
                        out=out_d[k, :, 0:G], in_=hn[64:128, 0:G])
                else:
                    # SWDGE store: keeps both HWDGE queues free for streaming
                    nc.gpsimd.dma_start(
                        out=out_d[k, :, 0:G], in_=hn[64:128, 0:G])

                woff += 128 * G * J

    nc.compile()
    return nc


_CACHE = {}


def _get_nc(wdt, mdt):
    key = (wdt, mdt)
    if key not in _CACHE:
        _CACHE[key] = build_nc(wdt, mdt)
    return _CACHE[key]


def _shards(x, state, Wx, Wh, b, wdt_np):
    x2 = np.asarray(x, np.float32).reshape(ITEMS, C)
    h2 = np.asarray(state, np.float32).reshape(ITEMS, H)
    b2 = np.asarray(b, np.float32).reshape(ITEMS, J)
    aux2 = np.ascontiguousarray(np.concatenate([x2, h2, b2], axis=1))
    wx2 = np.asarray(Wx).reshape(ITEMS, C, J)
    wh2 = np.asarray(Wh).reshape(ITEMS, H, J)
    w2 = np.concatenate([wx2, wh2], axis=1).astype(wdt_np)
    w2 = w2.reshape(NCORES, PER, 2 * C, J)
    aux3 = aux2.reshape(NCORES, PER, AUXW)
    maps = []
    for i in range(NCORES):
        # aux packed per chunk: [128 partitions, nchunk * 320]
        auxp = np.zeros((128, NCHUNK * AUXW), np.float32)
        s = 0
        for k, G in enumerate(CHUNKS):
            auxp[0:G, k * AUXW:(k + 1) * AUXW] = aux3[i, s:s + G]
            s += G
        # per chunk: [items, c, j] -> [c, item-in-chunk, j], flattened
        blocks = []
        s = 0
        for G in CHUNKS:
            blocks.append(w2[i, s:s + G].transpose(1, 0, 2).ravel())
            s += G
        maps.append({"aux": auxp, "wxh": np.concatenate(blocks)})
    return maps


def kernel(x, state, Wx, Wh, b, _trace=False, _wdt=BF16, _mdt=BF16):
    import ml_dtypes
    wdt_np = np.float32 if _wdt == F32 else ml_dtypes.bfloat16
    nc = _get_nc(_wdt, _mdt)
    in_maps = _shards(x, state, Wx, Wh, b, wdt_np)
    res = run_bass_kernel_spmd(nc, in_maps, list(range(NCORES)), trace=_trace)
    # out: [NCHUNK, H, GMAX] per core, chunk-packed -> [ITEMS, H]
    out = np.empty((ITEMS, H), np.float32)
    for i in range(NCORES):
        o = res.results[i]["out"]
        s = 0
        for k, G in enumerate(CHUNKS):
            out[i * PER + s:i * PER + s + G] = o[k, :, 0:G].T
            s += G
    ret = out.reshape(B, N, 1, H)
    if _trace:
        return ret, res
    return ret
